# revision 15
# baseline (speedup 1.0000x reference)
"""Trainium2 Bass kernel for nn_ExpertsChooseMaskedExpand (MoE routing).

Reference computes (per batch b):
    xd[e,c,j] = sum_t mask[t,e,c] * x[t,e,j]          (dispatch)
    y[e,c,o]  = sum_j xd[e,c,j] * w[e,o,j] + bias[o]  (expert GEMM)
    out[t,o]  = sum_{e,c} comb[t,e,c] * y[e,c,o]      (combine)

We use associativity to contract comb with xd first:
    z[t,e,j] = sum_c comb[t,e,c] * xd[e,c,j]
    out[t,o] = sum_{e,j} z[t,e,j] * w[e,o,j] + bias[o] * S[t],
    S[t] = sum_{e,c} comb[t,e,c]
which cuts FLOPs ~3.4x and never materializes y (B,E,C,O).

Sharding: 8 cores; core k handles batch b=k//2, token half h=k%2.
Dispatch needs the whole batch's tokens. Hybrid split:
  - experts 0-3: dispatch duplicated across the pair (full batch mask)
    -> xd complete locally, incrementally, with no communication.
  - experts 4-7: token-split partial dispatch + ONE pair-wise 512KB
    AllReduce(add). Measured behavior: collective transfers only get
    DMA bandwidth once the bulk input stream quiets, so the cc
    completes ~20us after the last mask byte; everything it gates is
    scheduled after work that is available locally.

Scheduling (the point of this kernel): the PE executes in program
order, so fill work is emitted exactly where the PE would otherwise
stall on DMA:
  - per dup expert e: dispatch(e) -> z(e, chunk0) (its comb tile is
    DMA'd right behind e's mask) -> a layer of combine pass-1 pieces
    for token chunk-group 0 accumulated into bf16 partials with
    vector adds (so a single expert's z is enough to make progress).
  - split-expert dispatch then streams; z(e0-3, later chunks) covers
    the collective's latency window.
  - post-gate loads (remaining comb tiles, weights for e4-7) sit
    behind the collective readback on the same DMA queue, so they
    cannot starve the collective's transfers.
  - combine: chunk-group 0 adds experts 4-7 on top of the partials
    (vector merge); other chunk-groups run full 8-expert psum chains.

All matmuls run in bf16 with fp32 PSUM accumulation; inputs are
cast/re-laid-out on host. The graph is SPMD-uniform.
"""

import numpy as np
import ml_dtypes

BF16 = ml_dtypes.bfloat16

B, T, E, C = 4, 4096, 8, 512
I = 128            # per-expert input features
O = 4096           # out_features
NCORES = 8
TLOC = B * T // NCORES      # 2048 tokens per core
NTT = T // 128              # 32 token tiles, full batch (dup dispatch)
NTTL = TLOC // 128          # 16 local token tiles (split dispatch)
NCT = C // 128              # 4 c-blocks
NTC = TLOC // 512           # 4 t-chunks (z stage)
NOT = O // 512              # 8 o-tiles

_CACHE = {}


def _build():
    import concourse.bass as bass
    import concourse.tile as tile
    import concourse.bacc as bacc
    import concourse.mybir as mybir

    f32 = mybir.dt.float32
    bf16 = mybir.dt.bfloat16
    ts = bass.ts
    add_op = mybir.AluOpType.add
    mult_op = mybir.AluOpType.mult

    nc = bacc.Bacc(None, target_bir_lowering=False, debug=False)

    xhd = nc.dram_tensor("xhd", [4, 128, NTT, I], bf16, kind="ExternalInput")
    mhd = nc.dram_tensor("mhd", [4, 128, NTT, C], bf16, kind="ExternalInput")
    xhs = nc.dram_tensor("xhs", [4, 128, NTTL, I], bf16,
                         kind="ExternalInput")
    mhs = nc.dram_tensor("mhs", [4, 128, NTTL, C], bf16,
                         kind="ExternalInput")
    cbt = nc.dram_tensor("cbt", [E, NCT, 128, TLOC], bf16,
                         kind="ExternalInput")
    wf = nc.dram_tensor("wf", [128, E, O], bf16, kind="ExternalInput")
    ident = nc.dram_tensor("ident", [128, 128], bf16, kind="ExternalInput")
    out_d = nc.dram_tensor("out", [TLOC, O], f32, kind="ExternalOutput")

    groups = [[0, 1], [2, 3], [4, 5], [6, 7]]

    with tile.TileContext(nc) as tc:
        with (
            tc.tile_pool(name="persist", bufs=1) as persist,
            tc.tile_pool(name="stream", bufs=1) as stream,
            tc.tile_pool(name="psum", bufs=1, space="PSUM") as psum,
            tc.tile_pool(name="dram", bufs=1, space="DRAM") as dram,
        ):
            wf_sb = persist.tile([128, E, O], bf16, tag="wf")
            id_sb = persist.tile([128, 128], bf16, tag="ident")
            nc.scalar.dma_start(id_sb[:], ident[:])

            cc_in = dram.tile([4, 128, NCT, 128], bf16, name="ccin")
            cc_out = dram.tile([4, 128, NCT, 128], bf16, name="ccout")

            xd = {}   # e -> xd tile [128c, NCT, 128j] bf16
            zt = {}   # (e, tch) -> z^T tile [128j, 512t] bf16
            po = {}   # (tt, ot) -> bf16 partial over experts 0..cur

            def dispatch(e, xsrc, msrc, ei, ntt, to_cc):
                ps_a = psum.tile([128, C], f32, tag="psA", bufs=2,
                                 name=f"psA{e}")
                for q0 in range(0, ntt, 8):
                    mh_t = stream.tile([128, 8, C], bf16, tag="mh", bufs=3,
                                       name=f"mh{e}_{q0}")
                    nc.sync.dma_start(mh_t[:], msrc[ei, :, q0:q0 + 8, :])
                    xh_t = stream.tile([128, 8, I], bf16, tag="xh", bufs=3,
                                       name=f"xh{e}_{q0}")
                    nc.scalar.dma_start(xh_t[:], xsrc[ei, :, q0:q0 + 8, :])
                    for i in range(8):
                        tt = q0 + i
                        nc.tensor.matmul(
                            ps_a[:],
                            xh_t[:, i, :],
                            mh_t[:, i, :],
                            start=(tt == 0),
                            stop=(tt == ntt - 1),
                        )
                xdt = stream.tile([128, C], bf16, tag="xdt", bufs=2,
                                  name=f"xdt{e}")
                nc.vector.tensor_copy(xdt[:], ps_a[:])
                xdp = stream.tile([128, NCT, 128], bf16, tag="xdp", bufs=2,
                                  name=f"xdp{e}") if to_cc else \
                    persist.tile([128, NCT, 128], bf16, tag=f"xd{e}",
                                 name=f"xd{e}")
                for cb in range(NCT):
                    ps_t = psum.tile([128, 128], bf16, tag="psT", bufs=2,
                                     name=f"psT{e}_{cb}")
                    nc.tensor.transpose(ps_t[:],
                                        xdt[:, ts(cb, 128)], id_sb[:])
                    nc.vector.tensor_copy(xdp[:, cb, :], ps_t[:])
                if to_cc:
                    nc.scalar.dma_start(cc_in[e - 4], xdp[:])
                else:
                    xd[e] = xdp

            def zstage(e, tch):
                cb_t = stream.tile([128, NCT, 512], bf16, tag="cb", bufs=3,
                                   name=f"cb{e}_{tch}")
                for cb in range(NCT):
                    nc.sync.dma_start(cb_t[:, cb, :],
                                      cbt[e, cb, :, ts(tch, 512)])
                ps_z = psum.tile([128, 512], f32, tag="psZ", bufs=2,
                                 name=f"psZ{e}_{tch}")
                for cb in range(NCT):
                    nc.tensor.matmul(
                        ps_z[:],
                        xd[e][:, cb, :],
                        cb_t[:, cb, :],
                        start=(cb == 0),
                        stop=(cb == NCT - 1),
                    )
                z_sb = persist.tile([128, 512], bf16, tag=f"zt{e}_{tch}",
                                    name=f"zt{e}_{tch}")
                nc.vector.tensor_copy(z_sb[:], ps_z[:])
                zt[(e, tch)] = z_sb

            def pass1_layer(e):
                """Add expert e's combine contribution for chunk-group 0
                into the bf16 partials (vector-accumulated so a single
                expert's z is enough to make PE progress)."""
                for tt in range(4):
                    for ot in range(NOT):
                        ps_p = psum.tile([128, 512], f32, tag="psC",
                                         bufs=2, name=f"psP{e}_{tt}_{ot}")
                        nc.tensor.matmul(
                            ps_p[:],
                            zt[(e, 0)][:, ts(tt, 128)],
                            wf_sb[:, e, ts(ot, 512)],
                            start=True, stop=True)
                        if e == 0:
                            pb = persist.tile(
                                [128, 512], bf16, tag=f"po{tt}_{ot}",
                                name=f"po{tt}_{ot}")
                            nc.vector.tensor_copy(pb[:], ps_p[:])
                            po[(tt, ot)] = pb
                        else:
                            pb = po[(tt, ot)]
                            nc.vector.scalar_tensor_tensor(
                                pb[:], ps_p[:], 1.0, pb[:],
                                mult_op, add_op)

            # ---- Own phase: dup experts with inline fill ----
            for e in range(4):
                nc.scalar.dma_start(wf_sb[:, e, :], wf[:, e, :])
                dispatch(e, xhd, mhd, e, NTT, False)
                zstage(e, 0)
                pass1_layer(e)

            # split experts stream; their partial xd goes to the cc
            for e in range(4, 8):
                dispatch(e, xhs, mhs, e - 4, NTTL, True)
            nc.gpsimd.collective_compute(
                "AllReduce",
                mybir.AluOpType.add,
                replica_groups=groups,
                ins=[cc_in[:].opt()],
                outs=[cc_out[:].opt()],
            )

            # cc-latency window fill: z for dup experts, chunk 1
            for e in range(4):
                zstage(e, 1)

            # gate: readback (blocks the sync queue until the cc lands,
            # so everything below it cannot starve the collective)
            for e in range(4, 8):
                xr = persist.tile([128, NCT, 128], bf16, tag=f"xd{e}",
                                  name=f"xd{e}")
                nc.sync.dma_start(xr[:], cc_out[e - 4])
                xd[e] = xr
            for e in range(4, 8):
                nc.sync.dma_start(wf_sb[:, e, :], wf[:, e, :])
            for tch in range(NTC):
                for e in range(4, 8):
                    zstage(e, tch)
                if tch >= 2:
                    for e in range(4):
                        zstage(e, tch)

            # ---- Combine phase ----
            for tt in range(NTTL):
                tch, m = tt // 4, tt % 4
                two_pass = tt < 4
                e0 = 4 if two_pass else 0
                out_sb = stream.tile([128, O // 2], f32, tag="out",
                                     bufs=2, name=f"out{tt}")
                for ot in range(NOT):
                    if ot == NOT // 2:
                        nc.scalar.dma_start(
                            out_d[ts(tt, 128), 0:O // 2], out_sb[:])
                        out_sb = stream.tile([128, O // 2], f32,
                                             tag="out", bufs=2,
                                             name=f"out{tt}b")
                    ps_c = psum.tile([128, 512], f32, tag="psC",
                                     bufs=2, name=f"psC{tt}_{ot}")
                    for e in range(e0, E):
                        nc.tensor.matmul(
                            ps_c[:],
                            zt[(e, tch)][:, ts(m, 128)],
                            wf_sb[:, e, ts(ot, 512)],
                            start=(e == e0),
                            stop=(e == E - 1),
                        )
                    dst = out_sb[:, ts(ot % 4, 512)]
                    if two_pass:
                        nc.vector.scalar_tensor_tensor(
                            dst, ps_c[:], 1.0, po[(tt, ot)][:],
                            mult_op, add_op)
                    else:
                        nc.vector.tensor_copy(dst, ps_c[:])
                nc.scalar.dma_start(
                    out_d[ts(tt, 128), O // 2:O], out_sb[:])

    nc.compile()
    return nc


def _prep_inputs(x, weight, bias, combine_array, dispatch_mask):
    """Host-side cast to bf16 + re-layout for contiguous device DMA."""
    x = np.asarray(x, np.float32)
    weight = np.asarray(weight, np.float32)
    bias = np.asarray(bias, np.float32)
    comb = np.asarray(combine_array, np.float32)
    mask = np.asarray(dispatch_mask, np.float32)

    xf = np.ascontiguousarray(
        x.reshape(B, NTT, 128, E, I).transpose(0, 3, 2, 1, 4)).astype(BF16)
    mf = np.ascontiguousarray(
        mask.reshape(B, NTT, 128, E, C).transpose(0, 3, 2, 1, 4)
    ).astype(BF16)
    xs = np.ascontiguousarray(
        x.reshape(B, 2, NTTL, 128, E, I).transpose(0, 1, 4, 3, 2, 5)
    ).astype(BF16)
    ms = np.ascontiguousarray(
        mask.reshape(B, 2, NTTL, 128, E, C).transpose(0, 1, 4, 3, 2, 5)
    ).astype(BF16)
    cbt = np.ascontiguousarray(
        comb.reshape(B, 2, TLOC, E, NCT, 128).transpose(0, 1, 3, 4, 5, 2)
    ).astype(BF16)
    wfh = np.ascontiguousarray(
        weight.reshape(E, O, I).transpose(2, 0, 1)).astype(BF16)
    s = comb.sum(axis=(2, 3))
    idm = np.eye(128, dtype=BF16)

    in_maps = []
    for k in range(NCORES):
        b, h = k // 2, k % 2
        in_maps.append({
            "xhd": xf[b, 0:4], "mhd": mf[b, 0:4],
            "xhs": xs[b, h, 4:8], "mhs": ms[b, h, 4:8],
            "cbt": cbt[b, h], "wf": wfh, "ident": idm,
        })
    return in_maps, s, bias


def kernel(x, weight, bias, combine_array, dispatch_mask):
    from concourse import bass_utils

    if "nc" not in _CACHE:
        _CACHE["nc"] = _build()
    nc = _CACHE["nc"]

    in_maps, s, bias_f = _prep_inputs(
        x, weight, bias, combine_array, dispatch_mask)
    res = bass_utils.run_bass_kernel_spmd(
        nc, in_maps, core_ids=list(range(NCORES)))
    out = np.stack([res.results[k]["out"] for k in range(NCORES)])
    out = out.reshape(B, T, O)
    out += s[:, :, None] * bias_f[None, None, :]
    return out.astype(np.float32)


# revision 22
# speedup vs baseline: 1.2510x; 1.2510x over previous
"""Trainium2 Bass kernel for nn_ExpertsChooseMaskedExpand (MoE routing).

Reference computes (per batch b):
    xd[e,c,j] = sum_t mask[t,e,c] * x[t,e,j]          (dispatch)
    y[e,c,o]  = sum_j xd[e,c,j] * w[e,o,j] + bias[o]  (expert GEMM)
    out[t,o]  = sum_{e,c} comb[t,e,c] * y[e,c,o]      (combine)

We use associativity to contract comb with xd first:
    z[t,e,j] = sum_c comb[t,e,c] * xd[e,c,j]
    out[t,o] = sum_{e,j} z[t,e,j] * w[e,o,j] + bias[o] * S[t],
    S[t] = sum_{e,c} comb[t,e,c]
which cuts FLOPs ~3.4x and never materializes y (B,E,C,O).

Sharding: 8 cores; core k handles batch b=k//2, token half h=k%2 (2048
tokens). Each core computes its batch's full xd locally (dispatch work
duplicated across the pair) so no cross-core communication is needed.

Phasing: the head phase (dispatch) is DMA-bound and the combine phase is
PE-bound, so the z-stage (B) is interleaved into the combine phase per
t-chunk: comb loads ride under combine matmuls instead of inflating the
head phase. All matmuls run in bf16 with fp32 PSUM accumulation; inputs
are cast and re-laid-out on the host so every DMA is wide and contiguous.
"""

import numpy as np
import ml_dtypes

BF16 = ml_dtypes.bfloat16

B, T, E, C = 4, 4096, 8, 512
I = 128            # per-expert input features
O = 4096           # out_features
NCORES = 8
TLOC = B * T // NCORES      # 2048 tokens per core
NTT = T // 128              # 32 token tiles per batch (dispatch)
NQ = 4                      # dispatch chunk groups (8 token-tiles each)
NCT = C // 128              # 4 c-tiles
NTC = TLOC // 512           # 4 t-chunks per core (z stage)
NOT = O // 512              # 8 o-tiles
NTTL = TLOC // 128          # 16 local token tiles (final stage)

_CACHE = {}


def _build():
    import concourse.bass as bass
    import concourse.tile as tile
    import concourse.bacc as bacc
    import concourse.mybir as mybir

    f32 = mybir.dt.float32
    bf16 = mybir.dt.bfloat16
    ts = bass.ts

    nc = bacc.Bacc(None, target_bir_lowering=False, debug=False)

    xh = nc.dram_tensor("xh", [E, 128, NTT, I], bf16, kind="ExternalInput")
    mh = nc.dram_tensor("mh", [E, 128, NTT, C], bf16, kind="ExternalInput")
    cbt = nc.dram_tensor("cbt", [E, NCT, 128, TLOC], bf16, kind="ExternalInput")
    wf = nc.dram_tensor("wf", [128, E, O], bf16, kind="ExternalInput")
    ident = nc.dram_tensor("ident", [128, 128], bf16, kind="ExternalInput")
    out_d = nc.dram_tensor("out", [TLOC, O], f32, kind="ExternalOutput")

    with tile.TileContext(nc) as tc:
        with (
            tc.tile_pool(name="persist", bufs=1) as persist,
            tc.tile_pool(name="psum", bufs=1, space="PSUM") as psum,
        ):
            wf_sb = persist.tile([128, E, O], bf16, tag="wf")
            id_sb = persist.tile([128, 128], bf16, tag="ident")
            nc.scalar.dma_start(id_sb[:], ident[:])

            xd = {}   # e -> xd tile [128c, (ct j)] bf16
            zt = {}   # (e, tc) -> z^T tile [128j, 512t] bf16

            def stage_b(e, tch, cb_pool, cb_bufs):
                # z^T[e][tch] = xd[e] (c x j) contracted with comb^T
                cb_t = cb_pool.tile([128, NCT, 512], bf16, tag="cb",
                                    bufs=cb_bufs, name=f"cb{e}_{tch}")
                for ct in range(NCT):
                    nc.sync.dma_start(cb_t[:, ct, :],
                                      cbt[e, ct, :, ts(tch, 512)])
                ps_b = psum.tile([128, 512], f32, tag="psB", bufs=2,
                                 name=f"psB{e}_{tch}")
                for ct in range(NCT):
                    nc.tensor.matmul(
                        ps_b[:],
                        xd[e][:, ts(ct, 128)],
                        cb_t[:, ct, :],
                        start=(ct == 0),
                        stop=(ct == NCT - 1),
                    )
                z_sb = persist.tile([128, 512], bf16, tag=f"zt{e}_{tch}",
                                    name=f"zt{e}_{tch}")
                nc.vector.tensor_copy(z_sb[:], ps_b[:])
                zt[(e, tch)] = z_sb

            # ---- Head phase: dispatch (DMA-bound) + B(tc=0) ----
            pout = {}  # (tt, ot) -> bf16 partial of combine over e=0..3
            pqueue = [(ptt, pot) for ptt in range(6) for pot in range(NOT)]

            with (
                tc.tile_pool(name="head", bufs=1) as head,
                tc.tile_pool(name="psumA", bufs=1, space="PSUM") as psum_a,
                tc.tile_pool(name="psumP", bufs=1, space="PSUM") as psum_p,
            ):
                for e in range(E):
                    # one PSUM bank per ct: start=True zeroes a whole 2KB
                    # zero region, so accumulation groups must not share one
                    ps_a = [psum_a.tile([128, 128], f32, tag="psA", bufs=4,
                                        name=f"psA{e}_{ct}") for ct in range(NCT)]
                    chunks = ([(0, 2), (2, 2), (4, 4)] if e == 0 else []) + \
                        [(q * 8, 8) for q in range(1 if e == 0 else 0, NQ)]
                    for q0, qn in chunks:
                        mh_t = head.tile([128, 8, C], bf16, tag="mh", bufs=4,
                                         name=f"mh{e}_{q0}")
                        nc.sync.dma_start(mh_t[:, 0:qn, :],
                                          mh[e, :, q0:q0 + qn, :])
                        xh_t = head.tile([128, 8, I], bf16, tag="xh", bufs=5,
                                         name=f"xh{e}_{q0}")
                        nc.scalar.dma_start(xh_t[:, 0:qn, :],
                                            xh[e, :, q0:q0 + qn, :])
                        for i in range(qn):
                            tt = q0 + i
                            for ct in range(NCT):
                                nc.tensor.matmul(
                                    ps_a[ct][:],
                                    mh_t[:, i, ts(ct, 128)],
                                    xh_t[:, i, :],
                                    start=(tt == 0),
                                    stop=(tt == NTT - 1),
                                )
                        if e >= E // 2 and pqueue:
                            for ptt, pot in [pqueue.pop(0) for _ in
                                             range(min(3, len(pqueue)))]:
                                ptc, pm = ptt // 4, ptt % 4
                                ps_p = psum_p.tile(
                                    [128, 512], f32, tag="psP", bufs=2,
                                    name=f"psP{ptt}_{pot}")
                                for pe in range(E // 2):
                                    nc.tensor.matmul(
                                        ps_p[:],
                                        zt[(pe, ptc)][:, ts(pm, 128)],
                                        wf_sb[:, pe, ts(pot, 512)],
                                        start=(pe == 0),
                                        stop=(pe == E // 2 - 1),
                                    )
                                po = persist.tile(
                                    [128, 512], bf16, tag=f"po{ptt}_{pot}",
                                    name=f"po{ptt}_{pot}")
                                nc.vector.tensor_copy(po[:], ps_p[:])
                                pout[(ptt, pot)] = po
                    # weight slice prefetch rides behind this expert's loads
                    nc.scalar.dma_start(wf_sb[:, e, :], wf[:, e, :])
                    xd_sb = persist.tile([128, C], bf16, tag="xd", bufs=8,
                                         name=f"xd{e}")
                    for ct in range(NCT):
                        nc.vector.tensor_copy(xd_sb[:, ts(ct, 128)], ps_a[ct][:])
                    xd[e] = xd_sb
                    stage_b(e, 0, head, 2)
                    if e < 4:
                        stage_b(e, 1, head, 2)

            # ---- Combine phase (PE-bound), stage B interleaved per tc ----
            with (
                tc.tile_pool(name="tail", bufs=1) as tail,
                tc.tile_pool(name="psumC", bufs=1, space="PSUM") as psum_c,
            ):
                for tcg in range(NTC):
                    for tt in range(tcg * 4, tcg * 4 + 4):
                        m = tt % 4
                        out_sb = tail.tile([128, O // 2], f32, tag="out",
                                           bufs=3, name=f"out{tt}")
                        for ot in range(NOT):
                            if ot == NOT // 2:
                                nc.scalar.dma_start(
                                    out_d[ts(tt, 128), 0:O // 2], out_sb[:])
                                out_sb = tail.tile([128, O // 2], f32,
                                                   tag="out", bufs=3,
                                                   name=f"out{tt}b")
                            ps_c = psum_c.tile([128, 512], f32, tag="psC",
                                               bufs=6, name=f"psC{tt}_{ot}")
                            two = tt < 6
                            e0 = E // 2 if two else 0
                            for e in range(e0, E):
                                nc.tensor.matmul(
                                    ps_c[:],
                                    zt[(e, tcg)][:, ts(m, 128)],
                                    wf_sb[:, e, ts(ot, 512)],
                                    start=(e == e0),
                                    stop=(e == E - 1),
                                )
                            dst = out_sb[:, ts(ot % 4, 512)]
                            if two:
                                # merge head-phase partial (experts 0-3)
                                # on the vector engine -- no PE cost
                                nc.vector.scalar_tensor_tensor(
                                    dst, ps_c[:], 1.0, pout[(tt, ot)][:],
                                    mybir.AluOpType.mult,
                                    mybir.AluOpType.add)
                            else:
                                nc.vector.tensor_copy(dst, ps_c[:])
                        nc.scalar.dma_start(
                            out_d[ts(tt, 128), O // 2:O], out_sb[:])
                    if tcg + 1 < NTC:
                        for e in range(E):
                            if tcg == 0 and e < 4:
                                continue  # z(e0-3, 1) done in the head
                            stage_b(e, tcg + 1, tail, 6)

    nc.compile()
    return nc


def _prep_inputs(x, weight, bias, combine_array, dispatch_mask):
    """Host-side cast to bf16 + re-layout for contiguous device DMA."""
    x = np.asarray(x, np.float32)
    weight = np.asarray(weight, np.float32)
    bias = np.asarray(bias, np.float32)
    comb = np.asarray(combine_array, np.float32)
    mask = np.asarray(dispatch_mask, np.float32)

    # xh[b]: (E, 128, NTT, I); xh[b][e, p, tt, j] = x[b, tt*128+p, e, j]
    xh = np.ascontiguousarray(
        x.reshape(B, NTT, 128, E, I).transpose(0, 3, 2, 1, 4)).astype(BF16)
    # mh[b]: (E, 128, NTT, C)
    mh = np.ascontiguousarray(
        mask.reshape(B, NTT, 128, E, C).transpose(0, 3, 2, 1, 4)).astype(BF16)
    # cbt[b][h]: (E, NCT, 128, TLOC);
    # [..., e, ct, p, t] = comb[b, h*TLOC+t, e, ct*128+p]
    cbt = np.ascontiguousarray(
        comb.reshape(B, 2, TLOC, E, NCT, 128).transpose(0, 1, 3, 4, 5, 2)
    ).astype(BF16)
    # wf: (128, E, O); wf[j, e, o] = weight.reshape(E, O, I)[e, o, j]
    wf = np.ascontiguousarray(
        weight.reshape(E, O, I).transpose(2, 0, 1)).astype(BF16)
    # S[b, t] = sum_{e,c} comb[b, t, e, c] -- bias*S added on host in f32
    s = comb.sum(axis=(2, 3))
    idm = np.eye(128, dtype=BF16)

    in_maps = []
    for k in range(NCORES):
        b, h = k // 2, k % 2
        in_maps.append({
            "xh": xh[b], "mh": mh[b], "cbt": cbt[b, h], "wf": wf,
            "ident": idm,
        })
    return in_maps, s, bias


def kernel(x, weight, bias, combine_array, dispatch_mask):
    from concourse import bass_utils

    if "nc" not in _CACHE:
        _CACHE["nc"] = _build()
    nc = _CACHE["nc"]

    in_maps, s, bias_f = _prep_inputs(
        x, weight, bias, combine_array, dispatch_mask)
    res = bass_utils.run_bass_kernel_spmd(
        nc, in_maps, core_ids=list(range(NCORES)))
    out = np.stack([res.results[k]["out"] for k in range(NCORES)])
    out = out.reshape(B, T, O)
    out += s[:, :, None] * bias_f[None, None, :]
    return out.astype(np.float32)

